# revision 2
# baseline (speedup 1.0000x reference)
# Multi-head causal attention (B=4, S=2048, D=1024, H=16) on 8 TRN2 NeuronCores.
#
# v4 sharding: batch x head-half. Core c handles batch b=c//2 and heads
# (c%2)*8 .. +8 (4 head-pairs) over the FULL sequence. Causal structure is
# therefore identical on every core: four 512-row query chunks with exact
# kk-tile caps (4, 8, 12, 16) -- no wasted zero tiles, masks only on the 4
# diagonal tiles of each chunk (same masks on all cores).
#
# Chunks are processed in order [1, 2, 3, 0]: chunk 1 needs only half the
# K/V tiles so the preamble stays small, later K/V tiles + the chunk-0 Q
# projection drain as PE filler work, and the tiny cap-4 chunk 0 lands last
# so the output-projection + collective tail is short.
#
# Output projection is computed as a per-core partial (contraction over the
# local 512 v-dims) with bias/2 folded in; partials are summed pairwise with
# an AllReduce over DRAM bounce buffers (one per q-chunk; each fires as soon
# as that chunk's partials are staged so the collectives overlap compute).
# The gpsimd queue carries ONLY the collectives + final DRAM copies; softmax
# denominators are broadcast with K=1 PE matmuls so no engine queue ever
# waits behind a collective. Host picks each core's q-half.
#
#   St[kk, q]: Kt[d, s], Qt[d, q]; St = Kt_tile.T @ Qt (2 heads packed into
#   one 2-bank PSUM tile, exp'd in a single ACT op)
#   P = exp(St) * tri_mask (diagonal tiles only)
#   OT[dv, q] += V_aug[kk, 65].T @ P  -- V carries a ones column, so PSUM
#     row 64 accumulates the softmax denominators for free.
import sys

if '/opt/trn_rl_repo' not in sys.path:
    sys.path.insert(0, '/opt/trn_rl_repo')

import numpy as np

B, S, D = 4, 2048, 1024
H, DK = 16, 64
NCORES = 8
SC = 512
NKT = S // 128            # 16 kk tiles
HPL = 4                   # local head-pairs (8 heads)
GROUPS = [[0, 1], [2, 3], [4, 5], [6, 7]]
CH_ORDER = [0, 1, 2, 3]

_CACHE = {}


def _build_program():
    import contextlib

    import concourse.tile as tile
    from concourse import bacc, mybir

    F32 = mybir.dt.float32
    BF16 = mybir.dt.bfloat16
    EXP = mybir.ActivationFunctionType.Exp

    nc = bacc.Bacc("TRN2", target_bir_lowering=False, debug=False,
                   num_devices=NCORES)

    xT_d = nc.dram_tensor("xT", [D, S], BF16, kind="ExternalInput")
    wqT_d = nc.dram_tensor("wqT", [D, 512], BF16, kind="ExternalInput")
    wkT_d = nc.dram_tensor("wkT", [D, 512], BF16, kind="ExternalInput")
    wvT_d = nc.dram_tensor("wvT", [D, 512], BF16, kind="ExternalInput")
    woT_d = nc.dram_tensor("woT", [512, D], BF16, kind="ExternalInput")
    bias_d = nc.dram_tensor("bias", [1, D], BF16, kind="ExternalInput")
    masks_d = nc.dram_tensor("masks", [128, 4 * 512], BF16,
                             kind="ExternalInput")
    y_d = nc.dram_tensor("y", [1024, D], BF16, kind="ExternalOutput")

    with tile.TileContext(nc) as tc, contextlib.ExitStack() as ctx:
        smalls = ctx.enter_context(tc.tile_pool(name="smalls", bufs=1))
        p_OT = ctx.enter_context(tc.tile_pool(name="otp", bufs=1))
        p_Kt = ctx.enter_context(tc.tile_pool(name="ktp", bufs=1))
        p_Qt = ctx.enter_context(tc.tile_pool(name="qtp", bufs=1))
        p_V = ctx.enter_context(tc.tile_pool(name="vp", bufs=1))
        p_mk = ctx.enter_context(tc.tile_pool(name="mk", bufs=1))
        p_x = ctx.enter_context(tc.tile_pool(name="xp", bufs=1))
        p_w = ctx.enter_context(tc.tile_pool(name="wp", bufs=1))
        p_yb = ctx.enter_context(tc.tile_pool(name="ybp", bufs=4))
        p_dram = ctx.enter_context(
            tc.tile_pool(name="dram", bufs=1, space="DRAM"))

        OT = p_OT.tile([128, HPL * S], BF16, tag="OT")
        Kt = p_Kt.tile([128, HPL * S], BF16, tag="Kt")
        Qt = p_Qt.tile([128, HPL * S], BF16, tag="Qt")
        Vsb = p_V.tile([128, NKT * HPL * 130], BF16, tag="Vsb")
        masks_sb = p_mk.tile([128, 4 * 512], BF16, tag="masks")
        xsb = p_x.tile([128, 8 * S], BF16, tag="xsb")
        wq = p_w.tile([128, 8 * 512], BF16, tag="wq")
        wk = p_w.tile([128, 8 * 512], BF16, tag="wk")
        wv = p_w.tile([128, 8 * 512], BF16, tag="wv")
        wo = p_w.tile([128, 4 * D], BF16, tag="wo")
        bias_sb = smalls.tile([1, D], BF16, tag="bias")
        bias_bc = smalls.tile([128, D], BF16, tag="bias_bc")
        ones1f = smalls.tile([1, 128], F32, tag="ones1f")
        nc.vector.memset(ones1f[:], 1.0)
        ones128f = smalls.tile([128, 128], F32, tag="ones128f")
        nc.vector.memset(ones128f[:], 1.0)

        # ones columns of V_aug (all 16 s-tiles x 8 local heads)
        nc.vector.tensor_copy(
            Vsb[:].rearrange("p (s h c) -> p s h c", s=NKT, c=65)
            [:, :, :, 64:65],
            ones128f[:].rearrange("p (s h) -> p s h", s=NKT)[:, :, :, None])

        # DRAM bounce buffers for the pairwise partial-y reduction
        ybounce = [p_dram.tile([SC, D], BF16, tag=f"ybi{ci}",
                               name=f"ybi{ci}") for ci in range(4)]
        obounce = [p_dram.tile([SC // 2, D], BF16, tag=f"ybo{ci}",
                               name=f"ybo{ci}") for ci in range(4)]

        def _fire_ar(ci):
            # pairwise ReduceScatter: rank 0 of the pair gets rows 0:256 of
            # the summed chunk, rank 1 rows 256:512 -- exactly the q rows
            # that core is responsible for. Host stitches the quarters.
            nc.gpsimd.collective_compute(
                "ReduceScatter", mybir.AluOpType.add,
                replica_groups=GROUPS,
                ins=[ybounce[ci].opt()], outs=[obounce[ci].opt()])
            nc.gpsimd.dma_start(y_d.ap()[ci * 256:(ci + 1) * 256, :],
                                obounce[ci][:])

        # ---------------- input DMAs, in need-order -----------------------
        for k in range(8):
            nc.sync.dma_start(wk[:, k * 512:(k + 1) * 512],
                              wkT_d.ap()[k * 128:(k + 1) * 128, :])
            nc.sync.dma_start(xsb[:, k * S:k * S + 512],
                              xT_d.ap()[k * 128:(k + 1) * 128, 0:512])
        for k in range(8):
            nc.sync.dma_start(wv[:, k * 512:(k + 1) * 512],
                              wvT_d.ap()[k * 128:(k + 1) * 128, :])
            nc.sync.dma_start(
                xsb[:, k * S + 512:k * S + 1024],
                xT_d.ap()[k * 128:(k + 1) * 128, 512:1024])
        nc.scalar.dma_start(masks_sb[:], masks_d.ap())
        nc.scalar.dma_start(bias_sb[:], bias_d.ap())
        for k in range(8):
            nc.sync.dma_start(wq[:, k * 512:(k + 1) * 512],
                              wqT_d.ap()[k * 128:(k + 1) * 128, :])
        for cb in range(2, 4):
            for k in range(8):
                nc.sync.dma_start(
                    xsb[:, k * S + cb * 512:k * S + (cb + 1) * 512],
                    xT_d.ap()[k * 128:(k + 1) * 128,
                              cb * 512:(cb + 1) * 512])
        for dc in range(4):
            nc.sync.dma_start(wo[:, dc * D:(dc + 1) * D],
                              woT_d.ap()[dc * 128:(dc + 1) * 128, :])

        # bias/2 broadcast for the out-proj partials (gpsimd; runs early,
        # long before the first collective lands on this queue)
        nc.gpsimd.partition_broadcast(bias_bc[:], bias_sb[:])

        # warmup collective: initializes CC channels and acts as a start
        # barrier so the real per-chunk reductions see minimal partner skew
        warm_i = p_dram.tile([1, 16], F32, tag="warm_i", name="warm_i")
        warm_o = p_dram.tile([1, 16], F32, tag="warm_o", name="warm_o")
        nc.sync.dma_start(warm_i[:], ones1f[0:1, 0:16])
        nc.gpsimd.collective_compute(
            "AllReduce", mybir.AluOpType.add,
            replica_groups=GROUPS,
            ins=[warm_i.opt()], outs=[warm_o.opt()])

        # ---------------- projection / out-proj unit emitters -------------
        def alloc_of(pool):
            def alloc(n, nm):
                return [pool.tile([128, 512], F32, tag="fx",
                                  name=f"{nm}_{i}")[:] for i in range(n)]
            return alloc

        def q_unit(hp, cis, alloc):
            def emit():
                ps = alloc(len(cis), f"psq_{hp}")
                for k in range(8):
                    for i, ci in enumerate(cis):
                        nc.tensor.matmul(
                            ps[i],
                            wq[:, k * 512 + hp * 128:k * 512 + (hp + 1) * 128],
                            xsb[:, k * S + ci * 512:k * S + (ci + 1) * 512],
                            start=(k == 0), stop=(k == 7))
                for i, ci in enumerate(cis):
                    nc.vector.tensor_copy(
                        Qt[:, hp * S + ci * 512:hp * S + (ci + 1) * 512],
                        ps[i])
            return emit

        def k_unit(hp, scs, alloc):
            def emit():
                ps = alloc(len(scs), f"psk_{hp}")
                for k in range(8):
                    for i, sc in enumerate(scs):
                        nc.tensor.matmul(
                            ps[i],
                            wk[:, k * 512 + hp * 128:k * 512 + (hp + 1) * 128],
                            xsb[:, k * S + sc * 512:k * S + (sc + 1) * 512],
                            start=(k == 0), stop=(k == 7))
                for i, sc in enumerate(scs):
                    nc.vector.tensor_copy(
                        Kt[:, hp * S + sc * 512:hp * S + (sc + 1) * 512],
                        ps[i])
            return emit

        def v_unit(st, alloc):
            # one s-tile, all 8 local heads: out ps [s 128, dv 512]
            def emit():
                ps = alloc(1, f"psv_{st}")[0]
                for k in range(8):
                    nc.tensor.matmul(
                        ps,
                        xsb[:, k * S + st * 128:k * S + (st + 1) * 128],
                        wv[:, k * 512:(k + 1) * 512],
                        start=(k == 0), stop=(k == 7))
                off = st * HPL * 130
                nc.vector.tensor_copy(
                    Vsb[:, off:off + 520]
                    .rearrange("p (h c) -> p h c", c=65)[:, :, 0:64],
                    ps.rearrange("p (h c) -> p h c", c=64))
            return emit

        def outproj_unit(qi, alloc):
            # partial y rows qi*128..+128: contraction over local 512 dv,
            # bias/2 folded in via the PSUM->SBUF add; DMA to DRAM bounce.
            def emit():
                ps = alloc(2, f"psy_{qi}")
                for dc in range(4):
                    for nc2 in range(2):
                        nc.tensor.matmul(
                            ps[nc2],
                            OT[:, dc * S + qi * 128:dc * S + (qi + 1) * 128],
                            wo[:, dc * D + nc2 * 512:dc * D + (nc2 + 1) * 512],
                            start=(dc == 0), stop=(dc == 3))
                yb = p_yb.tile([128, 1024], BF16, tag="yb")
                for nc2 in range(2):
                    nc.vector.tensor_add(
                        yb[:, nc2 * 512:(nc2 + 1) * 512], ps[nc2],
                        bias_bc[:, nc2 * 512:(nc2 + 1) * 512])
                ci = qi // 4
                r0 = (qi % 4) * 128
                nc.sync.dma_start(ybounce[ci][r0:r0 + 128, :], yb[:])
            return emit

        # ---------------- preamble: K/V s-tiles 0-3, Q chunk 0 ------------
        with tc.tile_pool(name="psq", bufs=4, space="PSUM") as psq:
            al = alloc_of(psq)
            k_unit(0, [0], al)()
            v_unit(0, al)()
            v_unit(1, al)()
            q_unit(0, [0], al)()
            v_unit(2, al)()
            v_unit(3, al)()
            k_unit(1, [0], al)()
            q_unit(1, [0], al)()
            k_unit(2, [0], al)()
            q_unit(2, [0], al)()
            k_unit(3, [0], al)()
            q_unit(3, [0], al)()

        # ---------------- attention ---------------------------------------
        def attn_block(ci, hp, cap, p_st, p_av, p_fx, pump, p_P, p_rs, p_bc):
            av = [p_av.tile([128, 512], F32, tag="av",
                            name=f"av_{ci}_{hp}_{hh}")
                  for hh in range(2)]

            def c0_of(t):
                # first causally-valid q column of diagonal tile t (128r of
                # the 512-wide chunk); earlier columns are fully masked and
                # skipped in the score/mask/AV ops. t==0 always writes the
                # full range so PSUM accumulation regions are initialized.
                if t < 4 * ci or t == 0:
                    return 0
                return 128 * (t - 4 * ci)

            def emit_av(t, p1):
                c0 = c0_of(t)
                for hh in range(2):
                    off = t * HPL * 130 + hp * 130 + hh * 65
                    nc.tensor.matmul(
                        av[hh][0:65, c0:512],
                        Vsb[:, off:off + 65],
                        p1[:, hh * 512 + c0:(hh + 1) * 512],
                        start=(t == 0), stop=(t == cap - 1))

            def emit_scores_exp(t):
                # ci==0 keeps full-width scores: its early st PSUM slots are
                # otherwise partially uninitialized under the exp read.
                c0 = c0_of(t) if ci > 0 else 0
                st = p_st.tile([128, 1024], F32, tag="st")
                for hh in range(2):
                    r0 = 64 * hh
                    nc.tensor.matmul(
                        st[:, hh * 512 + c0:(hh + 1) * 512],
                        Kt[r0:r0 + 64,
                           hp * S + t * 128:hp * S + (t + 1) * 128],
                        Qt[r0:r0 + 64,
                           hp * S + ci * 512 + c0:hp * S + (ci + 1) * 512],
                        start=True, stop=True,
                        tile_position=(r0, 0))
                p1 = p_P.tile([128, 1024], BF16, tag="p")
                nc.scalar.activation(p1[:], st[:], EXP)
                if t >= 4 * ci:
                    midx = t - 4 * ci
                    mc0 = c0_of(t)
                    p2 = p_P.tile([128, 1024], BF16, tag="p")
                    for hf in range(2):
                        nc.vector.tensor_mul(
                            p2[:, hf * 512 + mc0:(hf + 1) * 512],
                            p1[:, hf * 512 + mc0:(hf + 1) * 512],
                            masks_sb[:, midx * 512 + mc0:(midx + 1) * 512])
                    p1 = p2
                return p1

            lag = 3
            pending = []
            for t in range(cap):
                p1 = emit_scores_exp(t)
                if len(pending) > lag:
                    tt, pp = pending.pop(0)
                    emit_av(tt, pp)
                if pump:
                    pump(t)
                pending.append((t, p1))
            for tt, pp in pending:
                emit_av(tt, pp)
            # normalize: denominators broadcast via K=1 PE matmuls
            rs = p_rs.tile([1, 1024], F32, tag="rs")
            for hh in range(2):
                nc.vector.tensor_copy(
                    rs[0:1, hh * 512:hh * 512 + 512],
                    av[hh][64:65, :])
            rbc = p_bc.tile([128, 1024], F32, tag="rbc")
            scr = p_bc.tile([128, 1024], F32, tag="scr")
            bcs = [p_fx.tile([128, 512], F32, tag="fx",
                             name=f"bc_{ci}_{hp}_{hh}") for hh in range(2)]
            for hh in range(2):
                nc.tensor.matmul(
                    bcs[hh][:], ones1f[:],
                    rs[0:1, hh * 512:(hh + 1) * 512],
                    start=True, stop=True)
            for hh in range(2):
                nc.vector.reciprocal_approx_accurate(
                    rbc[:, hh * 512:(hh + 1) * 512], bcs[hh][:],
                    scratch=scr[:, hh * 512:(hh + 1) * 512])
            for hh in range(2):
                r0 = 64 * hh
                nc.vector.tensor_mul(
                    OT[r0:r0 + 64,
                       hp * S + ci * 512:hp * S + (ci + 1) * 512],
                    av[hh][0:64, :],
                    rbc[r0:r0 + 64, hh * 512:hh * 512 + 512])
            if pump:
                pump(None)

        fillers = {0: [], 1: [], 2: [], 3: []}

        def make_pump(ci, per_slot):
            frac = [0.0]

            def pump(t):
                if t is None:
                    n = 1
                else:
                    frac[0] += per_slot
                    n = int(frac[0])
                    frac[0] -= n
                for _ in range(n):
                    if fillers[ci]:
                        fillers[ci].pop(0)()
            return pump

        for ci in CH_ORDER:
            cap = 4 * ci + 4
            with tc.tile_pool(name=f"rs{ci}", bufs=1) as p_rs, \
                 tc.tile_pool(name=f"bcp{ci}", bufs=1) as p_bc, \
                 tc.tile_pool(name=f"pp{ci}", bufs=(8 if ci < 2 else 12)) \
                    as p_P, \
                 tc.tile_pool(name=f"pst{ci}", bufs=2, space="PSUM") as p_st, \
                 tc.tile_pool(name=f"pav{ci}", bufs=2, space="PSUM") as p_av, \
                 tc.tile_pool(name=f"pfx{ci}", bufs=2, space="PSUM") as p_fx:
                al = alloc_of(p_fx)

                if ci == 0:
                    fillers[0] += [q_unit(hp, [1], al) for hp in range(4)]
                    fillers[0] += [k_unit(hp, [1], al) for hp in range(4)]
                    fillers[0] += [v_unit(st, al) for st in range(4, 8)]
                elif ci == 1:
                    fillers[1] += [outproj_unit(qi, al) for qi in range(0, 4)]
                    fillers[1] += [lambda: _fire_ar(0)]
                    fillers[1] += [q_unit(hp, [2, 3], al) for hp in range(4)]
                    fillers[1] += [k_unit(hp, [2], al) for hp in range(4)]
                    fillers[1] += [v_unit(st, al) for st in range(8, 12)]
                elif ci == 2:
                    fillers[2] += [outproj_unit(qi, al) for qi in range(4, 8)]
                    fillers[2] += [lambda: _fire_ar(1)]
                    fillers[2] += [k_unit(hp, [3], al) for hp in range(4)]
                    fillers[2] += [v_unit(st, al) for st in range(12, 16)]
                else:
                    fillers[3] += [outproj_unit(qi, al) for qi in range(8, 12)]
                    fillers[3] += [lambda: _fire_ar(2)]

                n_units = len(fillers[ci])
                n_slots = cap * 4
                pump = make_pump(ci, per_slot=n_units / max(n_slots, 1))

                for hp in range(4):
                    attn_block(ci, hp, cap, p_st, p_av, p_fx, pump,
                               p_P, p_rs, p_bc)
                while fillers[ci]:
                    fillers[ci].pop(0)()

                if ci == 3:
                    for qi in range(12, 16):
                        outproj_unit(qi, al)()
                    _fire_ar(3)

    nc.compile()
    return nc


def _get_program():
    if 'nc' not in _CACHE:
        _CACHE['nc'] = _build_program()
    return _CACHE['nc']


def _tri_masks():
    p = np.arange(128)[:, None]
    f = np.arange(SC)[None, :]
    return [(p <= f - 128 * r).astype(np.float32) for r in range(4)]


def kernel(x, w_q, w_k, w_v, w_o, b_o):
    import ml_dtypes
    from concourse.bass_utils import run_bass_kernel_spmd

    BF = ml_dtypes.bfloat16
    x = np.asarray(x, dtype=np.float32)
    nc = _get_program()

    scale = np.float32(1.0 / np.sqrt(DK))
    wqT = np.ascontiguousarray(np.asarray(w_q, np.float32).T * scale)
    wkT = np.ascontiguousarray(np.asarray(w_k, np.float32).T)
    wvT = np.ascontiguousarray(np.asarray(w_v, np.float32).T)
    woT = np.ascontiguousarray(np.asarray(w_o, np.float32).T)
    bias_h = (np.asarray(b_o, np.float32) * 0.5)[None, :]

    tri = np.concatenate(_tri_masks(), axis=1).astype(BF)

    in_maps = []
    for c in range(NCORES):
        b = c // 2
        h0 = (c % 2) * 512
        in_maps.append({
            "xT": np.ascontiguousarray(x[b].T).astype(BF),
            "wqT": np.ascontiguousarray(wqT[:, h0:h0 + 512]).astype(BF),
            "wkT": np.ascontiguousarray(wkT[:, h0:h0 + 512]).astype(BF),
            "wvT": np.ascontiguousarray(wvT[:, h0:h0 + 512]).astype(BF),
            "woT": np.ascontiguousarray(woT[h0:h0 + 512, :]).astype(BF),
            "bias": bias_h.astype(BF),
            "masks": tri,
        })

    res = run_bass_kernel_spmd(nc, in_maps, core_ids=list(range(NCORES)),
                               trace=_CACHE.get('trace', False),
                               tmpdir=_CACHE.get('tmpdir'))
    _CACHE['last_res'] = res

    y = np.empty((B, S, D), dtype=np.float32)
    for c in range(NCORES):
        b = c // 2
        half = c % 2
        yc = np.asarray(res.results[c]["y"], dtype=np.float32)
        for ci in range(4):
            r0 = ci * 512 + half * 256
            y[b, r0:r0 + 256] = yc[ci * 256:(ci + 1) * 256]
    return y


# revision 3
# speedup vs baseline: 1.0569x; 1.0569x over previous
# Multi-head causal attention (B=4, S=2048, D=1024, H=16) on 8 TRN2 NeuronCores.
#
# v4 sharding: batch x head-half. Core c handles batch b=c//2 and heads
# (c%2)*8 .. +8 (4 head-pairs) over the FULL sequence. Causal structure is
# therefore identical on every core: four 512-row query chunks with exact
# kk-tile caps (4, 8, 12, 16) -- no wasted zero tiles, masks only on the 4
# diagonal tiles of each chunk (same masks on all cores).
#
# Chunks are processed in order [1, 2, 3, 0]: chunk 1 needs only half the
# K/V tiles so the preamble stays small, later K/V tiles + the chunk-0 Q
# projection drain as PE filler work, and the tiny cap-4 chunk 0 lands last
# so the output-projection + collective tail is short.
#
# Output projection is computed as a per-core partial (contraction over the
# local 512 v-dims) with bias/2 folded in; partials are summed pairwise with
# an AllReduce over DRAM bounce buffers (one per q-chunk; each fires as soon
# as that chunk's partials are staged so the collectives overlap compute).
# The gpsimd queue carries ONLY the collectives + final DRAM copies; softmax
# denominators are broadcast with K=1 PE matmuls so no engine queue ever
# waits behind a collective. Host picks each core's q-half.
#
#   St[kk, q]: Kt[d, s], Qt[d, q]; St = Kt_tile.T @ Qt (2 heads packed into
#   one 2-bank PSUM tile, exp'd in a single ACT op)
#   P = exp(St) * tri_mask (diagonal tiles only)
#   OT[dv, q] += V_aug[kk, 65].T @ P  -- V carries a ones column, so PSUM
#     row 64 accumulates the softmax denominators for free.
import sys

if '/opt/trn_rl_repo' not in sys.path:
    sys.path.insert(0, '/opt/trn_rl_repo')

import numpy as np

B, S, D = 4, 2048, 1024
H, DK = 16, 64
NCORES = 8
SC = 512
NKT = S // 128            # 16 kk tiles
HPL = 4                   # local head-pairs (8 heads)
GROUPS = [[0, 1], [2, 3], [4, 5], [6, 7]]
CH_ORDER = [0, 1, 2, 3]

_CACHE = {}


def _build_program():
    import contextlib

    import concourse.tile as tile
    from concourse import bacc, mybir

    F32 = mybir.dt.float32
    BF16 = mybir.dt.bfloat16
    EXP = mybir.ActivationFunctionType.Exp

    nc = bacc.Bacc("TRN2", target_bir_lowering=False, debug=False,
                   num_devices=NCORES)

    xT_d = nc.dram_tensor("xT", [D, S], BF16, kind="ExternalInput")
    wqT_d = nc.dram_tensor("wqT", [D, 512], BF16, kind="ExternalInput")
    wkT_d = nc.dram_tensor("wkT", [D, 512], BF16, kind="ExternalInput")
    wvT_d = nc.dram_tensor("wvT", [D, 512], BF16, kind="ExternalInput")
    woT_d = nc.dram_tensor("woT", [512, D], BF16, kind="ExternalInput")
    bias_d = nc.dram_tensor("bias", [1, D], BF16, kind="ExternalInput")
    masks_d = nc.dram_tensor("masks", [128, 4 * 512], BF16,
                             kind="ExternalInput")
    y_d = nc.dram_tensor("y", [1024, D], BF16, kind="ExternalOutput")

    with tile.TileContext(nc) as tc, contextlib.ExitStack() as ctx:
        smalls = ctx.enter_context(tc.tile_pool(name="smalls", bufs=1))
        p_OT = ctx.enter_context(tc.tile_pool(name="otp", bufs=1))
        p_Kt = ctx.enter_context(tc.tile_pool(name="ktp", bufs=1))
        p_Qt = ctx.enter_context(tc.tile_pool(name="qtp", bufs=1))
        p_V = ctx.enter_context(tc.tile_pool(name="vp", bufs=1))
        p_mk = ctx.enter_context(tc.tile_pool(name="mk", bufs=1))
        p_x = ctx.enter_context(tc.tile_pool(name="xp", bufs=1))
        p_w = ctx.enter_context(tc.tile_pool(name="wp", bufs=1))
        p_yb = ctx.enter_context(tc.tile_pool(name="ybp", bufs=4))
        p_dram = ctx.enter_context(
            tc.tile_pool(name="dram", bufs=1, space="DRAM"))

        OT = p_OT.tile([128, HPL * S], BF16, tag="OT")
        Kt = p_Kt.tile([128, HPL * S], BF16, tag="Kt")
        Qt = p_Qt.tile([128, HPL * S], BF16, tag="Qt")
        Vsb = p_V.tile([128, NKT * HPL * 130], BF16, tag="Vsb")
        masks_sb = p_mk.tile([128, 4 * 512], BF16, tag="masks")
        xsb = p_x.tile([128, 8 * S], BF16, tag="xsb")
        wq = p_w.tile([128, 8 * 512], BF16, tag="wq")
        wk = p_w.tile([128, 8 * 512], BF16, tag="wk")
        wv = p_w.tile([128, 8 * 512], BF16, tag="wv")
        wo = p_w.tile([128, 4 * D], BF16, tag="wo")
        bias_sb = smalls.tile([1, D], BF16, tag="bias")
        bias_bc = smalls.tile([128, D], BF16, tag="bias_bc")
        ones1f = smalls.tile([1, 128], F32, tag="ones1f")
        nc.vector.memset(ones1f[:], 1.0)
        ones128f = smalls.tile([128, 128], F32, tag="ones128f")
        nc.vector.memset(ones128f[:], 1.0)

        # ones columns of V_aug (all 16 s-tiles x 8 local heads)
        nc.vector.tensor_copy(
            Vsb[:].rearrange("p (s h c) -> p s h c", s=NKT, c=65)
            [:, :, :, 64:65],
            ones128f[:].rearrange("p (s h) -> p s h", s=NKT)[:, :, :, None])

        # DRAM bounce buffers for the pairwise partial-y reduction
        ybounce = [p_dram.tile([SC, D], BF16, tag=f"ybi{ci}",
                               name=f"ybi{ci}") for ci in range(4)]
        obounce = [p_dram.tile([SC // 2, D], BF16, tag=f"ybo{ci}",
                               name=f"ybo{ci}") for ci in range(4)]

        def _fire_ar(ci):
            # pairwise ReduceScatter: rank 0 of the pair gets rows 0:256 of
            # the summed chunk, rank 1 rows 256:512 -- exactly the q rows
            # that core is responsible for. Host stitches the quarters.
            nc.gpsimd.collective_compute(
                "ReduceScatter", mybir.AluOpType.add,
                replica_groups=GROUPS,
                ins=[ybounce[ci].opt()], outs=[obounce[ci].opt()])
            nc.gpsimd.dma_start(y_d.ap()[ci * 256:(ci + 1) * 256, :],
                                obounce[ci][:])

        # ---------------- input DMAs, in need-order -----------------------
        for k in range(8):
            nc.sync.dma_start(wk[:, k * 512:(k + 1) * 512],
                              wkT_d.ap()[k * 128:(k + 1) * 128, :])
            nc.sync.dma_start(xsb[:, k * S:k * S + 512],
                              xT_d.ap()[k * 128:(k + 1) * 128, 0:512])
        for k in range(8):
            nc.sync.dma_start(wv[:, k * 512:(k + 1) * 512],
                              wvT_d.ap()[k * 128:(k + 1) * 128, :])
            nc.sync.dma_start(
                xsb[:, k * S + 512:k * S + 1024],
                xT_d.ap()[k * 128:(k + 1) * 128, 512:1024])
        nc.scalar.dma_start(masks_sb[:], masks_d.ap())
        nc.scalar.dma_start(bias_sb[:], bias_d.ap())
        for k in range(8):
            nc.sync.dma_start(wq[:, k * 512:(k + 1) * 512],
                              wqT_d.ap()[k * 128:(k + 1) * 128, :])
        for cb in range(2, 4):
            for k in range(8):
                nc.sync.dma_start(
                    xsb[:, k * S + cb * 512:k * S + (cb + 1) * 512],
                    xT_d.ap()[k * 128:(k + 1) * 128,
                              cb * 512:(cb + 1) * 512])
        for dc in range(4):
            nc.sync.dma_start(wo[:, dc * D:(dc + 1) * D],
                              woT_d.ap()[dc * 128:(dc + 1) * 128, :])

        # bias/2 broadcast for the out-proj partials (gpsimd; runs early,
        # long before the first collective lands on this queue)
        nc.gpsimd.partition_broadcast(bias_bc[:], bias_sb[:])

        # warmup collective: initializes CC channels and acts as a start
        # barrier so the real per-chunk reductions see minimal partner skew
        warm_i = p_dram.tile([1, 16], F32, tag="warm_i", name="warm_i")
        warm_o = p_dram.tile([1, 16], F32, tag="warm_o", name="warm_o")
        nc.sync.dma_start(warm_i[:], ones1f[0:1, 0:16])
        nc.gpsimd.collective_compute(
            "AllReduce", mybir.AluOpType.add,
            replica_groups=GROUPS,
            ins=[warm_i.opt()], outs=[warm_o.opt()])

        # ---------------- projection / out-proj unit emitters -------------
        def alloc_of(pool):
            def alloc(n, nm):
                return [pool.tile([128, 512], F32, tag="fx",
                                  name=f"{nm}_{i}")[:] for i in range(n)]
            return alloc

        def q_unit(hp, cis, alloc):
            def emit():
                ps = alloc(len(cis), f"psq_{hp}")
                for k in range(8):
                    for i, ci in enumerate(cis):
                        nc.tensor.matmul(
                            ps[i],
                            wq[:, k * 512 + hp * 128:k * 512 + (hp + 1) * 128],
                            xsb[:, k * S + ci * 512:k * S + (ci + 1) * 512],
                            start=(k == 0), stop=(k == 7))
                for i, ci in enumerate(cis):
                    nc.vector.tensor_copy(
                        Qt[:, hp * S + ci * 512:hp * S + (ci + 1) * 512],
                        ps[i])
            return emit

        def k_unit(hp, scs, alloc):
            def emit():
                ps = alloc(len(scs), f"psk_{hp}")
                for k in range(8):
                    for i, sc in enumerate(scs):
                        nc.tensor.matmul(
                            ps[i],
                            wk[:, k * 512 + hp * 128:k * 512 + (hp + 1) * 128],
                            xsb[:, k * S + sc * 512:k * S + (sc + 1) * 512],
                            start=(k == 0), stop=(k == 7))
                for i, sc in enumerate(scs):
                    nc.vector.tensor_copy(
                        Kt[:, hp * S + sc * 512:hp * S + (sc + 1) * 512],
                        ps[i])
            return emit

        def v_unit(st, alloc):
            # one s-tile, all 8 local heads: out ps [s 128, dv 512]
            def emit():
                ps = alloc(1, f"psv_{st}")[0]
                for k in range(8):
                    nc.tensor.matmul(
                        ps,
                        xsb[:, k * S + st * 128:k * S + (st + 1) * 128],
                        wv[:, k * 512:(k + 1) * 512],
                        start=(k == 0), stop=(k == 7))
                off = st * HPL * 130
                nc.vector.tensor_copy(
                    Vsb[:, off:off + 520]
                    .rearrange("p (h c) -> p h c", c=65)[:, :, 0:64],
                    ps.rearrange("p (h c) -> p h c", c=64))
            return emit

        def outproj_unit(qi, alloc):
            # partial y rows qi*128..+128: contraction over local 512 dv,
            # bias/2 folded in via the PSUM->SBUF add; DMA to DRAM bounce.
            def emit():
                ps = alloc(2, f"psy_{qi}")
                for dc in range(4):
                    for nc2 in range(2):
                        nc.tensor.matmul(
                            ps[nc2],
                            OT[:, dc * S + qi * 128:dc * S + (qi + 1) * 128],
                            wo[:, dc * D + nc2 * 512:dc * D + (nc2 + 1) * 512],
                            start=(dc == 0), stop=(dc == 3))
                yb = p_yb.tile([128, 1024], BF16, tag="yb")
                for nc2 in range(2):
                    nc.vector.tensor_add(
                        yb[:, nc2 * 512:(nc2 + 1) * 512], ps[nc2],
                        bias_bc[:, nc2 * 512:(nc2 + 1) * 512])
                ci = qi // 4
                r0 = (qi % 4) * 128
                nc.sync.dma_start(ybounce[ci][r0:r0 + 128, :], yb[:])
            return emit

        # ---------------- preamble: K/V s-tiles 0-3, Q chunk 0 ------------
        with tc.tile_pool(name="psq", bufs=4, space="PSUM") as psq:
            al = alloc_of(psq)
            k_unit(0, [0], al)()
            v_unit(0, al)()
            v_unit(1, al)()
            q_unit(0, [0], al)()
            v_unit(2, al)()
            v_unit(3, al)()
            k_unit(1, [0], al)()
            q_unit(1, [0], al)()
            k_unit(2, [0], al)()
            q_unit(2, [0], al)()
            k_unit(3, [0], al)()
            q_unit(3, [0], al)()

        # ---------------- attention ---------------------------------------
        def attn_block(ci, hp, cap, p_st, p_av, p_fx, pump, p_P, p_rs, p_bc):
            av = [p_av.tile([128, 512], F32, tag="av",
                            name=f"av_{ci}_{hp}_{hh}")
                  for hh in range(2)]

            def c0_of(t):
                # first causally-valid q column of diagonal tile t (128r of
                # the 512-wide chunk); earlier columns are fully masked and
                # skipped in the score/mask/AV ops. t==0 always writes the
                # full range so PSUM accumulation regions are initialized.
                if t < 4 * ci or t == 0:
                    return 0
                return 128 * (t - 4 * ci)

            def emit_av(t, p1):
                c0 = c0_of(t)
                for hh in range(2):
                    off = t * HPL * 130 + hp * 130 + hh * 65
                    nc.tensor.matmul(
                        av[hh][0:65, c0:512],
                        Vsb[:, off:off + 65],
                        p1[:, hh * 512 + c0:(hh + 1) * 512],
                        start=(t == 0), stop=(t == cap - 1))

            def emit_scores_exp(t):
                # ci==0 keeps full-width scores: its early st PSUM slots are
                # otherwise partially uninitialized under the exp read.
                c0 = c0_of(t) if ci > 0 else 0
                st = p_st.tile([128, 1024], F32, tag="st")
                for hh in range(2):
                    r0 = 64 * hh
                    nc.tensor.matmul(
                        st[:, hh * 512 + c0:(hh + 1) * 512],
                        Kt[r0:r0 + 64,
                           hp * S + t * 128:hp * S + (t + 1) * 128],
                        Qt[r0:r0 + 64,
                           hp * S + ci * 512 + c0:hp * S + (ci + 1) * 512],
                        start=True, stop=True,
                        tile_position=(r0, 0))
                p1 = p_P.tile([128, 1024], BF16, tag="p")
                nc.scalar.activation(p1[:], st[:], EXP)
                if t >= 4 * ci:
                    midx = t - 4 * ci
                    mc0 = c0_of(t)
                    p2 = p_P.tile([128, 1024], BF16, tag="p")
                    for hf in range(2):
                        nc.vector.tensor_mul(
                            p2[:, hf * 512 + mc0:(hf + 1) * 512],
                            p1[:, hf * 512 + mc0:(hf + 1) * 512],
                            masks_sb[:, midx * 512 + mc0:(midx + 1) * 512])
                    p1 = p2
                return p1

            lag = 3
            pending = []
            for t in range(cap):
                p1 = emit_scores_exp(t)
                if len(pending) > lag:
                    tt, pp = pending.pop(0)
                    emit_av(tt, pp)
                if pump:
                    pump(t)
                pending.append((t, p1))
            for tt, pp in pending:
                emit_av(tt, pp)
            # normalize: denominators broadcast via K=1 PE matmuls
            rs = p_rs.tile([1, 1024], F32, tag="rs")
            for hh in range(2):
                nc.vector.tensor_copy(
                    rs[0:1, hh * 512:hh * 512 + 512],
                    av[hh][64:65, :])
            rbc = p_bc.tile([128, 1024], F32, tag="rbc")
            scr = p_bc.tile([128, 1024], F32, tag="scr")
            bcs = [p_fx.tile([128, 512], F32, tag="fx",
                             name=f"bc_{ci}_{hp}_{hh}") for hh in range(2)]
            for hh in range(2):
                nc.tensor.matmul(
                    bcs[hh][:], ones1f[:],
                    rs[0:1, hh * 512:(hh + 1) * 512],
                    start=True, stop=True)
            for hh in range(2):
                nc.vector.reciprocal_approx_accurate(
                    rbc[:, hh * 512:(hh + 1) * 512], bcs[hh][:],
                    scratch=scr[:, hh * 512:(hh + 1) * 512])
            for hh in range(2):
                r0 = 64 * hh
                nc.vector.tensor_mul(
                    OT[r0:r0 + 64,
                       hp * S + ci * 512:hp * S + (ci + 1) * 512],
                    av[hh][0:64, :],
                    rbc[r0:r0 + 64, hh * 512:hh * 512 + 512])
            if pump:
                pump(None)

        fillers = {0: [], 1: [], 2: [], 3: []}

        def make_pump(ci, per_slot):
            frac = [0.0]

            def pump(t):
                if t is None:
                    n = 1
                else:
                    frac[0] += per_slot
                    n = int(frac[0])
                    frac[0] -= n
                for _ in range(n):
                    if fillers[ci]:
                        fillers[ci].pop(0)()
            return pump

        for ci in CH_ORDER:
            cap = 4 * ci + 4
            with tc.tile_pool(name=f"rs{ci}", bufs=1) as p_rs, \
                 tc.tile_pool(name=f"bcp{ci}", bufs=1) as p_bc, \
                 tc.tile_pool(name=f"pp{ci}", bufs=(8 if ci < 2 else 12)) \
                    as p_P, \
                 tc.tile_pool(name=f"pst{ci}", bufs=2, space="PSUM") as p_st, \
                 tc.tile_pool(name=f"pav{ci}", bufs=2, space="PSUM") as p_av, \
                 tc.tile_pool(name=f"pfx{ci}", bufs=2, space="PSUM") as p_fx:
                al = alloc_of(p_fx)

                if ci == 0:
                    fillers[0] += [q_unit(hp, [1], al) for hp in range(4)]
                    fillers[0] += [k_unit(hp, [1], al) for hp in range(4)]
                    fillers[0] += [v_unit(st, al) for st in range(4, 8)]
                elif ci == 1:
                    fillers[1] += [outproj_unit(qi, al) for qi in range(0, 4)]
                    fillers[1] += [lambda: _fire_ar(0)]
                    fillers[1] += [q_unit(hp, [2, 3], al) for hp in range(4)]
                    fillers[1] += [k_unit(hp, [2, 3], al) for hp in range(4)]
                    fillers[1] += [v_unit(st, al) for st in range(8, 12)]
                elif ci == 2:
                    fillers[2] += [outproj_unit(qi, al) for qi in range(4, 8)]
                    fillers[2] += [lambda: _fire_ar(1)]
                    fillers[2] += [v_unit(st, al) for st in range(12, 16)]
                else:
                    fillers[3] += [outproj_unit(qi, al) for qi in range(8, 12)]
                    fillers[3] += [lambda: _fire_ar(2)]

                n_units = len(fillers[ci])
                n_slots = cap * 4
                pump = make_pump(ci, per_slot=n_units / max(n_slots, 1))

                for hp in range(4):
                    attn_block(ci, hp, cap, p_st, p_av, p_fx, pump,
                               p_P, p_rs, p_bc)
                while fillers[ci]:
                    fillers[ci].pop(0)()

                if ci == 3:
                    for qi in range(12, 16):
                        outproj_unit(qi, al)()
                    _fire_ar(3)

    nc.compile()
    return nc


def _get_program():
    if 'nc' not in _CACHE:
        _CACHE['nc'] = _build_program()
    return _CACHE['nc']


def _tri_masks():
    p = np.arange(128)[:, None]
    f = np.arange(SC)[None, :]
    return [(p <= f - 128 * r).astype(np.float32) for r in range(4)]


def kernel(x, w_q, w_k, w_v, w_o, b_o):
    import ml_dtypes
    from concourse.bass_utils import run_bass_kernel_spmd

    BF = ml_dtypes.bfloat16
    x = np.asarray(x, dtype=np.float32)
    nc = _get_program()

    scale = np.float32(1.0 / np.sqrt(DK))
    wqT = np.ascontiguousarray(np.asarray(w_q, np.float32).T * scale)
    wkT = np.ascontiguousarray(np.asarray(w_k, np.float32).T)
    wvT = np.ascontiguousarray(np.asarray(w_v, np.float32).T)
    woT = np.ascontiguousarray(np.asarray(w_o, np.float32).T)
    bias_h = (np.asarray(b_o, np.float32) * 0.5)[None, :]

    tri = np.concatenate(_tri_masks(), axis=1).astype(BF)

    in_maps = []
    for c in range(NCORES):
        b = c // 2
        h0 = (c % 2) * 512
        in_maps.append({
            "xT": np.ascontiguousarray(x[b].T).astype(BF),
            "wqT": np.ascontiguousarray(wqT[:, h0:h0 + 512]).astype(BF),
            "wkT": np.ascontiguousarray(wkT[:, h0:h0 + 512]).astype(BF),
            "wvT": np.ascontiguousarray(wvT[:, h0:h0 + 512]).astype(BF),
            "woT": np.ascontiguousarray(woT[h0:h0 + 512, :]).astype(BF),
            "bias": bias_h.astype(BF),
            "masks": tri,
        })

    res = run_bass_kernel_spmd(nc, in_maps, core_ids=list(range(NCORES)),
                               trace=_CACHE.get('trace', False),
                               tmpdir=_CACHE.get('tmpdir'))
    _CACHE['last_res'] = res

    y = np.empty((B, S, D), dtype=np.float32)
    for c in range(NCORES):
        b = c // 2
        half = c % 2
        yc = np.asarray(res.results[c]["y"], dtype=np.float32)
        for ci in range(4):
            r0 = ci * 512 + half * 256
            y[b, r0:r0 + 256] = yc[ci * 256:(ci + 1) * 256]
    return y
